# revision 1
# baseline (speedup 1.0000x reference)
"""Trainium2 kernel for nn_BackgroundNoiseLayer.

Computation (see reference):
  spikes[t,u] = noise_u[t,u] < 0.25                       (500 x 100, binary)
  W[n,u,r]    = scatter-add of bkg_weights[e]*factors[e,r] at (row[e], col[e])
  out[t, n*5+r] = sum_u W[n,u,r] * spikes[t,u]            (500 x 327680)

Sharding: neurons split 8192/core across 8 NeuronCores; spikes replicated.

Host side (index/layout prep): coalesce the sparse COO into the dense
per-core weight matrix Wc[u, n_local*5+r] (pure scatter of input products;
0.01% of total FLOPs), split into bf16 hi/lo halves so the device matmul
runs at full bf16 rate with ~1e-5 relative error (hi+lo accumulate in
fp32 PSUM).

Device side (per core): compute spikes from noise on DVE, then
out[t, :] = spikesT.T @ (W_hi + W_lo) via PE matmuls (K=100), copy
PSUM->SBUF on DVE/ACT, DMA the 82MB/core output to HBM.
"""

import sys

sys.path.insert(0, "/opt/trn_rl_repo")

import numpy as np
import ml_dtypes

import concourse.bacc as bacc
import concourse.tile as tile
import concourse.mybir as mybir
from concourse.bass_utils import run_bass_kernel_spmd

N_NEURONS = 65536
N_BKG = 100
R = 5
T = 500
NCORES = 8
NLOC = N_NEURONS // NCORES          # 8192 neurons per core
WCOLS = NLOC * R                    # 40960 free-dim columns per core
SPIKE_P = np.float32(250 * 0.001)   # 0.25

TCH = 125                           # t-chunk (4 chunks of 125 = 500)
NT = 512                            # matmul free-dim tile
GW = 2048                           # staging / W-group width (4 n-tiles)
NGRP = WCOLS // GW                  # 20 groups

_BF16 = mybir.dt.bfloat16
_F32 = mybir.dt.float32


def _build(reps=1, do_mm=True, do_copy=True, do_store=True, do_wload=True,
           gw=GW, alt_dma=False, stage_bufs=3, psum_w=NT, single_w=False,
           psum_bufs=None, fp32r=False, memset_stage=False,
           gpsimd_stores=False, dve_frac=2, t_outer=False):
    ngrp = WCOLS // gw
    _F32R = mybir.dt.float32r
    wdt = _F32R if fp32r else _BF16
    nc = bacc.Bacc("TRN2", target_bir_lowering=False, debug=False,
                   num_devices=NCORES)
    noise_t = nc.dram_tensor("noise_t", [N_BKG, T], _F32, kind="ExternalInput")
    if fp32r:
        w_hi = nc.dram_tensor("w_hi", [N_BKG, WCOLS], _F32R,
                              kind="ExternalInput")
        w_lo = None
        single_w = True
    else:
        w_hi = nc.dram_tensor("w_hi", [N_BKG, WCOLS], _BF16,
                              kind="ExternalInput")
        w_lo = None
        if not single_w:
            w_lo = nc.dram_tensor("w_lo", [N_BKG, WCOLS], _BF16,
                                  kind="ExternalInput")
    out = nc.dram_tensor("out", [T, WCOLS], _F32, kind="ExternalOutput")

    dma_engs = [nc.sync, nc.scalar] if alt_dma else [nc.sync]
    dma_i = 0

    def dma_eng():
        nonlocal dma_i
        e = dma_engs[dma_i % len(dma_engs)]
        dma_i += 1
        return e

    with tile.TileContext(nc) as tc:
        if psum_bufs is None:
            psum_bufs_ = 8 * NT // psum_w
        else:
            psum_bufs_ = psum_bufs
        with tc.tile_pool(name="const", bufs=1) as cpool, \
             tc.tile_pool(name="wpool", bufs=1) as wpool, \
             tc.tile_pool(name="stage", bufs=stage_bufs) as spool, \
             tc.tile_pool(name="psum", bufs=psum_bufs_, space="PSUM") as ppool:
            # spikes: load noise (u x t), compare, cast to bf16
            nz = cpool.tile([N_BKG, T], _F32, tag="noise")
            nc.sync.dma_start(nz[:], noise_t[:, :])
            sp32 = cpool.tile([N_BKG, T], _F32, tag="sp32")
            nc.vector.tensor_scalar(sp32[:], nz[:], float(SPIKE_P), None,
                                    mybir.AluOpType.is_lt)
            spk = cpool.tile([N_BKG, T], wdt, tag="spk")
            nc.vector.tensor_copy(spk[:], sp32[:])

            # W resident in SBUF, loaded in NGRP groups so matmuls overlap
            wh = []
            wl = []
            for g in range(ngrp):
                th = wpool.tile([N_BKG, gw], wdt, tag=f"wh{g}")
                if do_wload:
                    dma_eng().dma_start(th[:], w_hi[:, g * gw:(g + 1) * gw])
                wh.append(th)
                if w_lo is not None:
                    tl = wpool.tile([N_BKG, gw], _BF16, tag=f"wl{g}")
                    if do_wload:
                        dma_eng().dma_start(tl[:],
                                            w_lo[:, g * gw:(g + 1) * gw])
                    wl.append(tl)

            dummy = None
            if do_store and not do_copy:
                dummy = cpool.tile([128, gw], _F32, tag="dummy")
                nc.vector.memset(dummy[:], 0.0)

            copy_eng = 0
            for _rep in range(reps):
              order = ([(g, tci) for g in range(ngrp) for tci in range(4)]
                       if not t_outer else
                       [(g, tci) for tci in range(4) for g in range(ngrp)])
              for g, tci in order:
                    t0 = tci * TCH
                    lhs = spk[:, t0:t0 + TCH]
                    stg = (dummy if dummy is not None
                           else spool.tile([128, gw], _F32, tag="stage"))
                    if memset_stage:
                        nc.gpsimd.memset(stg[:], 0.0)
                    for jj in range(0 if memset_stage else gw // psum_w):
                        ps = ppool.tile([128, psum_w], _F32, tag="ps")
                        nsub = psum_w // NT
                        for js in range(nsub):
                            j = jj * nsub + js
                            pslice = ps[:TCH, js * NT:(js + 1) * NT]
                            if do_mm:
                                nc.tensor.matmul(
                                    pslice, lhs,
                                    wh[g][:, j * NT:(j + 1) * NT],
                                    start=True, stop=single_w)
                                if not single_w:
                                    nc.tensor.matmul(
                                        pslice, lhs,
                                        wl[g][:, j * NT:(j + 1) * NT],
                                        start=False, stop=True)
                        if do_copy:
                            dst = stg[:TCH, jj * psum_w:(jj + 1) * psum_w]
                            if copy_eng < dve_frac:
                                nc.vector.tensor_copy(dst, ps[:TCH, :])
                            else:
                                nc.scalar.copy(dst, ps[:TCH, :])
                            copy_eng = (copy_eng + 1) % 4
                    if do_store:
                        se = nc.gpsimd if gpsimd_stores else dma_eng()
                        se.dma_start(
                            out[t0:t0 + TCH, g * gw:(g + 1) * gw],
                            stg[:TCH, :])
    nc.compile()
    return nc


_cached = None


def _get_nc():
    global _cached
    if _cached is None:
        _cached = _build()
    return _cached


def _prep_inputs(noise_u, bkg_weights, factors, row_idx, col_idx):
    noise = np.ascontiguousarray(
        np.asarray(noise_u, dtype=np.float32).reshape(T, N_BKG).T)
    w = np.asarray(bkg_weights, dtype=np.float32)
    f = np.asarray(factors, dtype=np.float32)
    rows = np.asarray(row_idx).astype(np.int64)
    cols = np.asarray(col_idx).astype(np.int64)

    vals = w[:, None] * f                      # (nnz, R)
    cell = rows * N_BKG + cols                 # dense cell id
    ncells = N_NEURONS * N_BKG
    Wd = np.empty((ncells, R), dtype=np.float32)
    for r in range(R):
        Wd[:, r] = np.bincount(cell, weights=vals[:, r].astype(np.float64),
                               minlength=ncells)
    Wd = Wd.reshape(N_NEURONS, N_BKG, R)

    in_maps = []
    for c in range(NCORES):
        Wc = Wd[c * NLOC:(c + 1) * NLOC]                   # (NLOC, U, R)
        Wc = np.ascontiguousarray(
            Wc.transpose(1, 0, 2)).reshape(N_BKG, WCOLS)   # (U, NLOC*R)
        w_hi = Wc.astype(ml_dtypes.bfloat16)
        w_lo = (Wc - w_hi.astype(np.float32)).astype(ml_dtypes.bfloat16)
        in_maps.append({"noise_t": noise, "w_hi": w_hi, "w_lo": w_lo})
    return in_maps


def _run(in_maps, trace=False):
    nc = _get_nc()
    return run_bass_kernel_spmd(nc, in_maps, core_ids=list(range(NCORES)),
                                trace=trace)


def bench_exec_ns(in_maps, iters=32, warmup=4):
    """Median-free steady-state wall time per NEFF execution across the
    8-core mesh, measured by pipelining `iters` chained executions (outputs
    donated back as the next call's output buffers) with inputs resident on
    device. NTFF profiling is unavailable under this axon client, so this
    is the HW exec time proxy: it includes NEFF dispatch but no host
    transfers."""
    import time
    import jax
    import numpy as jnp_np
    from jax.sharding import Mesh, PartitionSpec
    from jax.experimental.shard_map import shard_map
    from concourse import bass2jax, mybir as _mb

    nc = _get_nc()
    bass2jax.install_neuronx_cc_hook()

    partition_name = (nc.partition_id_tensor.name
                      if nc.partition_id_tensor else None)
    in_names, out_names, out_avals, zero_outs = [], [], [], []
    for alloc in nc.m.functions[0].allocations:
        if not isinstance(alloc, _mb.MemoryLocationSet):
            continue
        name = alloc.memorylocations[0].name
        if alloc.kind == "ExternalInput":
            if name != partition_name:
                in_names.append(name)
        elif alloc.kind == "ExternalOutput":
            out_names.append(name)
            shape = tuple(alloc.tensor_shape)
            dtype = _mb.dt.np(alloc.dtype)
            out_avals.append(jax.core.ShapedArray(shape, dtype))
            zero_outs.append(np.zeros(shape, dtype))
    n_params = len(in_names)
    n_outs = len(out_avals)
    all_in_names = list(in_names) + list(out_names)
    if partition_name is not None:
        all_in_names = all_in_names + [partition_name]

    def _body(*args):
        operands = list(args)
        if partition_name is not None:
            operands.append(bass2jax.partition_id_tensor())
        outs = bass2jax._bass_exec_p.bind(
            *operands,
            out_avals=tuple(out_avals),
            in_names=tuple(all_in_names),
            out_names=tuple(out_names),
            lowering_input_output_aliases=(),
            sim_require_finite=True,
            sim_require_nnan=True,
            nc=nc,
        )
        return tuple(outs)

    devices = jax.devices()[:NCORES]
    mesh = Mesh(jnp_np.asarray(devices), ("core",))
    in_specs = (PartitionSpec("core"),) * (n_params + n_outs)
    out_specs = (PartitionSpec("core"),) * n_outs
    donate = tuple(range(n_params, n_params + n_outs))
    f = jax.jit(
        shard_map(_body, mesh=mesh, in_specs=in_specs, out_specs=out_specs,
                  check_rep=False),
        donate_argnums=donate, keep_unused=True)

    per_core = [[np.asarray(m[nm]) for nm in in_names] for m in in_maps]
    concat_in = [np.concatenate([per_core[c][i] for c in range(NCORES)], axis=0)
                 for i in range(n_params)]
    concat_zeros = [np.zeros((NCORES * z.shape[0], *z.shape[1:]), z.dtype)
                    for z in zero_outs]
    sharding = jax.sharding.NamedSharding(mesh, PartitionSpec("core"))
    dev_in = [jax.device_put(x, sharding) for x in concat_in]
    outs = tuple(jax.device_put(z, sharding) for z in concat_zeros)

    for _ in range(warmup):
        outs = f(*dev_in, *outs)
    jax.block_until_ready(outs)
    t0 = time.perf_counter()
    for _ in range(iters):
        outs = f(*dev_in, *outs)
    jax.block_until_ready(outs)
    t1 = time.perf_counter()
    return (t1 - t0) / iters * 1e9


def kernel(noise_u, bkg_weights, factors, row_idx, col_idx):
    in_maps = _prep_inputs(noise_u, bkg_weights, factors, row_idx, col_idx)
    res = _run(in_maps)
    out = np.concatenate([res.results[c]["out"] for c in range(NCORES)],
                         axis=1)
    return out.reshape(1, T, N_NEURONS * R).astype(np.float32, copy=False)

